# revision 1
# baseline (speedup 1.0000x reference)
"""Trainium2 Bass kernel for nn_EnhancedInformerAutoformerHybrid (sparse attention).

Contract: kernel(**inputs) takes FULL inputs (x [8,4096,1024] + QKVO weights),
returns FULL output [8,4096,1024]. Internally: data-parallel over batch across
8 NeuronCores (one batch per core), no collectives.

Math per core (batch b):
  Q = x@Wq.T+bq (fp32), K^T = Wk@x^T+bk (bf16), V = x@Wv.T+bv (bf16)
  per-(h,l) score = 0.5*||q|| + 0.3*entropy(softmax(q)) + 0.2*var(q)
  u = clip(round(std/mean*10), 3, 10) from head 0 scores
  top-10 l per (h) via two-stage max8/max_index; slots >= u dropped at scatter
  attention for selected rows only; out = bo broadcast + scatter of
  collision-merged (Eq-matmul) projected rows.

Precision (host-validated): Q fp32 (top-k margin ~1e-3 >> fp32 noise; bf16
would flip selections), everything downstream bf16 -> rel err ~7e-4.
"""

import os

import numpy as np

import concourse.bass as bass
import concourse.tile as tile
from concourse import bacc, mybir
from concourse.bass import ds, ts
from concourse.bass_utils import run_bass_kernel_spmd
from concourse.masks import make_identity

P = 128
B, L_FULL, D = 8, 4096, 1024
H, DK = 16, 64
KMAX = 10
NSEL = H * KMAX  # 160
NA, NB = 120, 40  # head groups 0-11 / 12-15
BIG = 1.0e9
F32 = mybir.dt.float32
BF16 = mybir.dt.bfloat16
I32 = mybir.dt.int32
AX = mybir.AxisListType
OP = mybir.AluOpType
ACT = mybir.ActivationFunctionType

LAST_RESULTS = None  # test.py reads profiling info from here
DEBUG_DUMPS = False
PHASE = 99  # stop-after-phase gate for HW bisection


def build_kernel(nc, tc, L):
    LC = L // P
    LC5 = L // 512
    QBLK = (LC * H) // P
    KD = D // P
    assert L % 1024 == 0 and (LC * H) % P == 0

    xt = nc.dram_tensor("xt", [D, L], F32, kind="ExternalInput").ap()
    wqt = nc.dram_tensor("wqt", [D, D], F32, kind="ExternalInput").ap()
    wkt = nc.dram_tensor("wkt", [D, D], F32, kind="ExternalInput").ap()
    wvt = nc.dram_tensor("wvt", [D, D], F32, kind="ExternalInput").ap()
    wot = nc.dram_tensor("wot", [D, D], F32, kind="ExternalInput").ap()
    bq = nc.dram_tensor("bq", [1, D], F32, kind="ExternalInput").ap()
    bkc = nc.dram_tensor("bkc", [P, KD], F32, kind="ExternalInput").ap()
    bv = nc.dram_tensor("bv", [1, D], F32, kind="ExternalInput").ap()
    bo = nc.dram_tensor("bo", [1, D], F32, kind="ExternalInput").ap()
    rowbase = nc.dram_tensor("rowbase", [P, 1], F32, kind="ExternalInput").ap()
    hvec16 = nc.dram_tensor("hvec16", [H, 1], F32, kind="ExternalInput").ap()
    slotA = nc.dram_tensor("slotA", [NA, 1], F32, kind="ExternalInput").ap()
    slotB = nc.dram_tensor("slotB", [NB, 1], F32, kind="ExternalInput").ap()

    out = nc.dram_tensor("out", [L, D], F32, kind="ExternalOutput").ap()

    q_bf = nc.dram_tensor("q_bf", [L, D], BF16).ap()
    cndv_d = nc.dram_tensor("cndv_d", [P, 16 * ((L // P * H) // P)], F32).ap()
    cndl_d = nc.dram_tensor("cndl_d", [P, 16 * ((L // P * H) // P)], F32).ap()
    topl_d = nc.dram_tensor("topl_d", [H, KMAX], F32).ap()
    grow_d = nc.dram_tensor("grow_d", [H, KMAX], F32).ap()
    kt_bf = nc.dram_tensor("kt_bf", [D, L], BF16).ap()
    v_bf = nc.dram_tensor("v_bf", [L, D], BF16).ap()

    xt_v = xt.rearrange("(ko p) l -> p ko l", p=P)

    with tc.tile_pool(name="pconst", bufs=1) as pconst, \
         tc.tile_pool(name="pbig", bufs=1) as pbig:

        ident_f = pconst.tile([P, P], F32)
        make_identity(nc, ident_f[:])
        ident_b = pconst.tile([P, P], BF16)
        make_identity(nc, ident_b[:])
        ones_1row = pconst.tile([1, P], F32)
        nc.vector.memset(ones_1row[:], 1.0)
        ones_col = pconst.tile([P, 1], F32)
        nc.vector.memset(ones_col[:], 1.0)

        rowbase_sb = pconst.tile([P, 1], F32)
        nc.sync.dma_start(rowbase_sb[:], rowbase[:])
        hvec_sb = pconst.tile([H, 1], F32)
        nc.sync.dma_start(hvec_sb[:], hvec16[:])
        slotA_sb = pconst.tile([NA, 1], F32)
        nc.sync.dma_start(slotA_sb[:], slotA[:])
        slotB_sb = pconst.tile([NB, 1], F32)
        nc.sync.dma_start(slotB_sb[:], slotB[:])
        bkc_sb = pconst.tile([P, KD], F32)
        nc.sync.dma_start(bkc_sb[:], bkc[:])

        # bf16 weights resident for the whole kernel (6MB)
        wkt_sb = pbig.tile([P, KD, D], BF16)
        wvt_sb = pbig.tile([P, KD, D], BF16)
        wot_sb = pbig.tile([P, KD, D], BF16)
        with tc.tile_pool(name="pwstage", bufs=1) as pwstage:
            for w_dram, w_sb in ((wkt, wkt_sb), (wvt, wvt_sb), (wot, wot_sb)):
                stg = pwstage.tile([P, KD, D], F32, tag="wstage")
                wv_ = w_dram.rearrange("(ko p) n -> p ko n", p=P)
                for k in range(KD):
                    nc.sync.dma_start(stg[:, k, :], wv_[:, k, :])
                    nc.gpsimd.tensor_copy(w_sb[:, k, :], stg[:, k, :])

        # bias broadcast tiles [128, D] (rows replicated) via ones-matmul
        bqb = pbig.tile([P, D], F32)
        bvb = pbig.tile([P, D], F32)
        bob = pbig.tile([P, D], F32)
        with tc.tile_pool(name="psb", bufs=2, space="PSUM") as psb:
            for b_dram, bb in ((bq, bqb), (bv, bvb), (bo, bob)):
                b1 = pconst.tile([1, D], F32, tag="b1")
                nc.sync.dma_start(b1[:], b_dram[:])
                for n in range(D // 512):
                    pbt = psb.tile([P, 512], F32, tag="bias_ps")
                    nc.tensor.matmul(pbt[:], lhsT=ones_1row[:], rhs=b1[:, ts(n, 512)],
                                     start=True, stop=True)
                    nc.vector.tensor_copy(bb[:, ts(n, 512)], pbt[:])

        sq_acc = pconst.tile([P, LC, H], F32)
        sm_acc = pconst.tile([P, LC, H], F32)
        se_acc = pconst.tile([P, LC, H], F32)
        seq_acc = pconst.tile([P, LC, H], F32)

        # ---- phase 1: Q (fp32) + V (bf16) projections, fused over l-chunks ----
        with tc.tile_pool(name="pwq", bufs=1) as pwq, \
             tc.tile_pool(name="st1", bufs=3) as st1, \
             tc.tile_pool(name="ps1", bufs=2, space="PSUM") as ps1:
            wqt_sb = pwq.tile([P, KD, D], F32)
            wqt_v = wqt.rearrange("(ko p) n -> p ko n", p=P)
            for k in range(KD):
                nc.sync.dma_start(wqt_sb[:, k, :], wqt_v[:, k, :])
            for lc in range(LC):
                xf = st1.tile([P, KD, P], F32, tag="xf")
                nc.sync.dma_start(xf[:], xt_v[:, :, ts(lc, P)])
                xb = st1.tile([P, KD, P], BF16, tag="xb")
                nc.gpsimd.tensor_copy(xb[:], xf[:])

                q_sb = st1.tile([P, D], F32, tag="q_sb")
                v_sb = st1.tile([P, D], BF16, tag="v_sb")
                for n in range(2):
                    pq = ps1.tile([P, 512], F32, tag="pq")
                    for k in range(KD):
                        nc.tensor.matmul(pq[:], lhsT=xf[:, k, :],
                                         rhs=wqt_sb[:, k, ts(n, 512)],
                                         start=(k == 0), stop=(k == KD - 1))
                    nc.vector.tensor_tensor(q_sb[:, ts(n, 512)], pq[:],
                                            bqb[:, ts(n, 512)], OP.add)
                    pv = ps1.tile([P, 512], F32, tag="pv")
                    for k in range(KD):
                        nc.tensor.matmul(pv[:], lhsT=xb[:, k, :],
                                         rhs=wvt_sb[:, k, ts(n, 512)],
                                         start=(k == 0), stop=(k == KD - 1))
                    nc.vector.tensor_tensor(v_sb[:, ts(n, 512)], pv[:],
                                            bvb[:, ts(n, 512)], OP.add)
                nc.sync.dma_start(v_bf[ts(lc, P), :], v_sb[:])
                qb_sb = st1.tile([P, D], BF16, tag="qb_sb")
                nc.gpsimd.tensor_copy(qb_sb[:], q_sb[:])
                nc.sync.dma_start(q_bf[ts(lc, P), :], qb_sb[:])

                tmp = st1.tile([P, D], F32, tag="stat_tmp")
                e_sb = st1.tile([P, D], F32, tag="e_sb")
                q3 = q_sb[:].rearrange("p (h k) -> p h k", k=DK)
                nc.vector.tensor_reduce(sm_acc[:, lc, :], q3, AX.X, OP.add)
                nc.vector.tensor_tensor(tmp[:], q_sb[:], q_sb[:], OP.mult)
                nc.vector.tensor_reduce(sq_acc[:, lc, :],
                                        tmp[:].rearrange("p (h k) -> p h k", k=DK),
                                        AX.X, OP.add)
                nc.scalar.activation(e_sb[:], q_sb[:], ACT.Exp)
                nc.vector.tensor_reduce(se_acc[:, lc, :],
                                        e_sb[:].rearrange("p (h k) -> p h k", k=DK),
                                        AX.X, OP.add)
                nc.vector.tensor_tensor(tmp[:], e_sb[:], q_sb[:], OP.mult)
                nc.vector.tensor_reduce(seq_acc[:, lc, :],
                                        tmp[:].rearrange("p (h k) -> p h k", k=DK),
                                        AX.X, OP.add)

        if PHASE <= 2:
            return
        # ---- phase 3: qs = 0.5*l2 + 0.3*entropy + 0.2*var  [128, LC*16] ----
        NF = LC * H
        qs = pconst.tile([P, NF], F32)
        sqf = sq_acc[:].rearrange("p c h -> p (c h)")
        smf = sm_acc[:].rearrange("p c h -> p (c h)")
        sef = se_acc[:].rearrange("p c h -> p (c h)")
        seqf = seq_acc[:].rearrange("p c h -> p (c h)")
        with tc.tile_pool(name="st3", bufs=1) as st3:
            t1 = st3.tile([P, NF], F32, tag="t1")
            t2 = st3.tile([P, NF], F32, tag="t2")
            nc.vector.tensor_tensor(t1[:], smf, smf, OP.mult)
            nc.vector.tensor_scalar_mul(t1[:], t1[:], 1.0 / DK)
            nc.vector.tensor_tensor(t1[:], sqf, t1[:], OP.subtract)
            nc.vector.tensor_scalar_mul(qs[:], t1[:], 0.2 / (DK - 1))
            nc.scalar.activation(t1[:], sqf, ACT.Sqrt)
            nc.vector.tensor_scalar_mul(t1[:], t1[:], 0.5)
            nc.vector.tensor_tensor(qs[:], qs[:], t1[:], OP.add)
            nc.scalar.activation(t1[:], sef, ACT.Ln)
            rc_se = st3.tile([P, NF], F32, tag="rc_se")
            nc.vector.reciprocal(rc_se[:], sef)
            nc.vector.tensor_tensor(t2[:], seqf, rc_se[:], OP.mult)
            nc.vector.tensor_tensor(t1[:], t1[:], t2[:], OP.subtract)
            nc.vector.tensor_scalar_mul(t1[:], t1[:], 0.3)
            nc.vector.tensor_tensor(qs[:], qs[:], t1[:], OP.add)

            # ---- phase 4: u = clip(round(std/mean*10), 3, 10), head 0 ----
            h0 = qs[:].rearrange("p (c h) -> p c h", h=H)[:, :, 0]
            s0 = st3.tile([P, 1], F32, tag="s0")
            g0 = st3.tile([P, 1], F32, tag="g0")
            t0 = st3.tile([P, LC], F32, tag="t0")
            nc.vector.tensor_reduce(s0[:], h0, AX.X, OP.add)
            nc.vector.tensor_tensor(t0[:], h0, h0, OP.mult)
            nc.vector.tensor_reduce(g0[:], t0[:], AX.X, OP.add)
            S1 = st3.tile([1, 4], F32, tag="S1")
            u_t = st3.tile([1, 1], F32, tag="u_t")
            with tc.tile_pool(name="ps4", bufs=1, space="PSUM") as ps4:
                pS = ps4.tile([1, 1], F32, tag="pS")
                nc.tensor.matmul(pS[:], lhsT=s0[:], rhs=ones_col[:], start=True, stop=True)
                pQ = ps4.tile([1, 1], F32, tag="pQ")
                nc.tensor.matmul(pQ[:], lhsT=g0[:], rhs=ones_col[:], start=True, stop=True)
                nc.vector.tensor_copy(S1[:, 0:1], pS[:])
                nc.vector.tensor_copy(S1[:, 1:2], pQ[:])
                nc.vector.tensor_tensor(S1[:, 2:3], S1[:, 0:1], S1[:, 0:1], OP.mult)
                nc.vector.tensor_scalar_mul(S1[:, 2:3], S1[:, 2:3], 1.0 / L)
                nc.vector.tensor_tensor(S1[:, 2:3], S1[:, 1:2], S1[:, 2:3], OP.subtract)
                nc.vector.tensor_scalar_mul(S1[:, 2:3], S1[:, 2:3], 1.0 / (L - 1))
                nc.scalar.activation(S1[:, 2:3], S1[:, 2:3], ACT.Sqrt)
                nc.vector.tensor_scalar(S1[:, 3:4], S1[:, 0:1], 1.0 / L, 1e-6,
                                        op0=OP.mult, op1=OP.add)
                rcm = st3.tile([1, 1], F32, tag="rcm")
                nc.vector.reciprocal(rcm[:], S1[:, 3:4])
                nc.vector.tensor_tensor(u_t[:], S1[:, 2:3], rcm[:], OP.mult)
                nc.vector.tensor_scalar_mul(u_t[:], u_t[:], 10.0)
                # u = clip(round(val), 3, 10); slot<u  <=>  slot<3 or val-slot>=0.5
                pU = ps4.tile([P, 1], F32, tag="pU")
                nc.tensor.matmul(pU[:], lhsT=ones_1row[:], rhs=u_t[:],
                                 start=True, stop=True)
                ub = pconst.tile([P, 1], F32)
                nc.vector.tensor_copy(ub[:], pU[:])

        if PHASE <= 4:
            return
        # ---- phase 5: two-stage top-k (values via max8, l via max_index) ----
        qsT = pconst.tile([P, QBLK, P], F32)
        with tc.tile_pool(name="ps5", bufs=2, space="PSUM") as ps5:
            for i in range(QBLK):
                pt = ps5.tile([P, P], F32, tag="pt_qs")
                nc.tensor.transpose(pt[:], qs[:, ts(i, P)], ident_f[:])
                nc.vector.tensor_copy(qsT[:, i, :], pt[:])
        NCW = 16 * QBLK                 # candidates per partition-row
        NCND = (P // H) * NCW           # candidates per head

        candv = pconst.tile([P, NCW], F32)
        candl = pconst.tile([P, NCW], F32)
        candv_h = pconst.tile([H, NCND], F32)
        candl_h = pconst.tile([H, NCND], F32)
        top_l = pconst.tile([H, KMAX], F32)
        grow = pconst.tile([H, KMAX], F32)
        with tc.tile_pool(name="st5", bufs=2) as st5:
            for bq_ in range(QBLK):
                blk = qsT[:, bq_, :]
                r1v = st5.tile([P, 8], F32, tag="r1v")
                r1i = st5.tile([P, 8], mybir.dt.uint32, tag="r1i")
                nc.vector.max(r1v[:], blk)
                nc.vector.max_index(r1i[:], r1v[:], blk)
                zap = st5.tile([P, P], F32, tag="zap")
                nc.vector.match_replace(zap[:], in_to_replace=r1v[:], in_values=blk,
                                        imm_value=0.0)
                r2v = st5.tile([P, 8], F32, tag="r2v")
                r2i = st5.tile([P, 8], mybir.dt.uint32, tag="r2i")
                nc.vector.max(r2v[:], zap[:])
                nc.vector.max_index(r2i[:], r2v[:], zap[:])
                nc.vector.tensor_copy(candv[:, ds(bq_ * 16, 8)], r1v[:])
                nc.vector.tensor_copy(candv[:, ds(bq_ * 16 + 8, 8)], r2v[:])
                fi = st5.tile([P, 16], F32, tag="fi")
                nc.vector.tensor_copy(fi[:, 0:8], r1i[:])
                nc.vector.tensor_copy(fi[:, 8:16], r2i[:])
                # l = idx + rowbase[p] + 1024*block
                nc.vector.tensor_scalar(candl[:, ds(bq_ * 16, 16)], fi[:],
                                        rowbase_sb[:, 0:1], float(1024 * bq_),
                                        op0=OP.add, op1=OP.add)

            nc.sync.dma_start(cndv_d[:], candv[:])
            nc.sync.dma_start(cndl_d[:], candl[:])
            nc.sync.dma_start(
                candv_h[:].rearrange("h (c s) -> h c s", s=NCW),
                cndv_d.rearrange("(c h) s -> h c s", h=H))
            nc.sync.dma_start(
                candl_h[:].rearrange("h (c s) -> h c s", s=NCW),
                cndl_d.rearrange("(c h) s -> h c s", h=H))

            b1v = st5.tile([H, 8], F32, tag="b1v")
            nc.vector.max(b1v[:], candv_h[:])
            zapB = st5.tile([H, NCND], F32, tag="zapB")
            nc.vector.match_replace(zapB[:], in_to_replace=b1v[:],
                                    in_values=candv_h[:], imm_value=0.0)
            b2v = st5.tile([H, 8], F32, tag="b2v")
            nc.vector.max(b2v[:], zapB[:])
            topv = st5.tile([H, KMAX], F32, tag="topv")
            nc.vector.tensor_copy(topv[:, 0:8], b1v[:])
            nc.vector.tensor_copy(topv[:, 8:KMAX], b2v[:, 0:2])

            eqm = st5.tile([H, NCND], F32, tag="eqm")
            msk = st5.tile([H, NCND], F32, tag="msk")
            for s in range(KMAX):
                nc.vector.tensor_tensor(eqm[:], candv_h[:],
                                        topv[:, s:s + 1].to_broadcast([H, NCND]),
                                        OP.is_equal)
                nc.vector.tensor_tensor(msk[:], eqm[:], candl_h[:], OP.mult)
                nc.vector.tensor_scalar(eqm[:], eqm[:], 1.0, -BIG,
                                        op0=OP.subtract, op1=OP.mult)
                nc.vector.tensor_tensor(msk[:], msk[:], eqm[:], OP.add)
                nc.vector.tensor_reduce(top_l[:, s:s + 1], msk[:], AX.X, OP.min)

            nc.vector.tensor_scalar_mul(grow[:], top_l[:], float(H))
            nc.vector.tensor_scalar(grow[:], grow[:], hvec_sb[:, 0:1], None,
                                    op0=OP.add)

        # rearrange [16,10] -> [160] partition-major (p = h*10+u)
        lA = pconst.tile([NA, 1], F32)
        lB = pconst.tile([NB, 1], F32)
        gA = pconst.tile([NA, 1], F32)
        gB = pconst.tile([NB, 1], F32)
        nc.sync.dma_start(topl_d[:], top_l[:])
        nc.sync.dma_start(grow_d[:], grow[:])
        topl_f = topl_d.rearrange("h s -> (h s)")
        grow_f = grow_d.rearrange("h s -> (h s)")
        nc.sync.dma_start(lA[:], topl_f[0:NA, None])
        nc.sync.dma_start(lB[:], topl_f[NA:NSEL, None])
        nc.sync.dma_start(gA[:], grow_f[0:NA, None])
        nc.sync.dma_start(gB[:], grow_f[NA:NSEL, None])
        gAi = pconst.tile([NA, 1], I32)
        nc.vector.tensor_copy(gAi[:], gA[:])
        gBi = pconst.tile([NB, 1], I32)
        nc.vector.tensor_copy(gBi[:], gB[:])

        def scatter_idx(l_t, slot_t, rows, name):
            f = pconst.tile([rows, 1], F32, tag=f"sf_{name}")
            v = pconst.tile([rows, 1], F32, tag=f"sv_{name}")
            va = pconst.tile([rows, 1], F32, tag=f"va_{name}")
            vb2 = pconst.tile([rows, 1], F32, tag=f"vb2_{name}")
            nc.vector.tensor_tensor(va[:], ub[:rows, :], slot_t[:], OP.subtract)
            nc.vector.tensor_scalar(va[:], va[:], 0.5, None, op0=OP.is_ge)
            nc.vector.tensor_scalar(vb2[:], slot_t[:], 3.0, None, op0=OP.is_lt)
            nc.vector.tensor_tensor(v[:], va[:], vb2[:], OP.max)
            nc.vector.tensor_scalar(f[:], l_t[:], float(L), None, op0=OP.subtract)
            nc.vector.tensor_tensor(f[:], f[:], v[:], OP.mult)
            nc.vector.tensor_scalar(f[:], f[:], float(L), None, op0=OP.add)
            it = pconst.tile([rows, 1], I32, tag=f"si_{name}")
            nc.vector.tensor_copy(it[:], f[:])
            return f, it

        sfA, siA = scatter_idx(lA, slotA_sb, NA, "A")
        sfB, siB = scatter_idx(lB, slotB_sb, NB, "B")

        if PHASE <= 5:
            return
        # ---- phase 2: K projection (bf16) into KT layout ----
        with tc.tile_pool(name="pwk", bufs=1) as pwk, \
             tc.tile_pool(name="st2", bufs=3) as st2, \
             tc.tile_pool(name="ps2", bufs=2, space="PSUM") as ps2:
            for lc5 in range(LC5):
                kxf = st2.tile([P, KD, 512], F32, tag="kxf")
                kxb = st2.tile([P, KD, 512], BF16, tag="kxb")
                for k in range(KD):
                    nc.sync.dma_start(kxf[:, k, :], xt_v[:, k, ts(lc5, 512)])
                    nc.gpsimd.tensor_copy(kxb[:, k, :], kxf[:, k, :])
                for do in range(KD):
                    pk = ps2.tile([P, 512], F32, tag="pk")
                    for k in range(KD):
                        nc.tensor.matmul(pk[:], lhsT=wkt_sb[:, k, ts(do, P)],
                                         rhs=kxb[:, k, :],
                                         start=(k == 0), stop=(k == KD - 1))
                    kt_sb = st2.tile([P, 512], BF16, tag="kt_sb")
                    nc.vector.tensor_tensor(
                        kt_sb[:], pk[:],
                        bkc_sb[:, do:do + 1].to_broadcast([P, 512]), OP.add)
                    nc.sync.dma_start(kt_bf[ts(do, P), ts(lc5, 512)], kt_sb[:])

        # ---- phase 6: gather Q rows -> QsT [64, 160] bf16 ----
        qsT_att = pconst.tile([DK, NSEL], BF16)
        with tc.tile_pool(name="st6", bufs=1) as st6, \
             tc.tile_pool(name="ps6", bufs=1, space="PSUM") as ps6:
            qgA = st6.tile([NA, DK], BF16, tag="qgA")
            nc.gpsimd.indirect_dma_start(
                out=qgA[:], out_offset=None,
                in_=q_bf.rearrange("l (h k) -> (l h) k", k=DK),
                in_offset=bass.IndirectOffsetOnAxis(ap=gAi[:, :1], axis=0))
            qgB = st6.tile([NB, DK], BF16, tag="qgB")
            nc.gpsimd.indirect_dma_start(
                out=qgB[:], out_offset=None,
                in_=q_bf.rearrange("l (h k) -> (l h) k", k=DK),
                in_offset=bass.IndirectOffsetOnAxis(ap=gBi[:, :1], axis=0))
            pga = ps6.tile([DK, NA], BF16, tag="pga")
            nc.tensor.transpose(pga[:], qgA[:], ident_b[:NA, :NA])
            nc.vector.tensor_copy(qsT_att[:, 0:NA], pga[:])
            pgb = ps6.tile([DK, NB], BF16, tag="pgb")
            nc.tensor.transpose(pgb[:], qgB[:], ident_b[:NB, :NB])
            nc.vector.tensor_copy(qsT_att[:, NA:NSEL], pgb[:])

        if PHASE <= 1:
            return
        # base output: out[l, :] = bo
        out_rows = out.rearrange("(c p) n -> p c n", p=P)
        for c in range(LC):
            nc.sync.dma_start(out_rows[:, c, :], bob[:])

        if PHASE <= 0:
            return
        if PHASE <= 6:
            return
        with tc.tile_pool(name="pbig2", bufs=1) as pbig2:
            # ---- phase 6b: block-diagonal Qblk [128, KD, 160] bf16 ----
            qblk = pbig2.tile([P, KD, NSEL], BF16)
            nc.vector.memset(qblk[:], 0.0)
            for h in range(H):
                nc.sync.dma_start(
                    qblk[ds((h % 2) * DK, DK), h // 2, ds(h * KMAX, KMAX)],
                    qsT_att[:, ds(h * KMAX, KMAX)])

            # ---- phase 7: scoresT[l, col] -> exp -> attn (no transposes) ----
            # scT = KT.T @ Qblk per l-chunk; softmax over l via ones-matmul sums
            eAll = pbig2.tile([P, LC, NSEL], F32)
            attnT = pbig2.tile([P, LC, NSEL], BF16)
            kt_v = kt_bf.rearrange("(dc p) l -> p dc l", p=P)
            S_sb = pconst.tile([1, NSEL], F32)
            rS = pconst.tile([1, NSEL], F32)
            rSb = pconst.tile([P, NSEL], F32)
            with tc.tile_pool(name="st7", bufs=3) as st7, \
                 tc.tile_pool(name="ps7", bufs=2, space="PSUM") as ps7, \
                 tc.tile_pool(name="ps7s", bufs=1, space="PSUM") as ps7s:
                psum_sum = ps7s.tile([1, NSEL], F32, tag="psum_sum")
                for lc in range(LC):
                    ktl = st7.tile([P, KD, P], BF16, tag="ktl")
                    nc.sync.dma_start(ktl[:], kt_v[:, :, ts(lc, P)])
                    ps_s = ps7.tile([P, NSEL], F32, tag="ps_s")
                    for dc in range(KD):
                        nc.tensor.matmul(ps_s[:], lhsT=ktl[:, dc, :], rhs=qblk[:, dc, :],
                                         start=(dc == 0), stop=(dc == KD - 1))
                    nc.scalar.activation(eAll[:, lc, :], ps_s[:], ACT.Exp, scale=0.125)
                    nc.tensor.matmul(psum_sum[:], lhsT=ones_col[:], rhs=eAll[:, lc, :],
                                     start=(lc == 0), stop=(lc == LC - 1))
                nc.vector.tensor_copy(S_sb[:], psum_sum[:])
                nc.vector.reciprocal(rS[:], S_sb[:])
                ps_b = ps7.tile([P, NSEL], F32, tag="ps_b")
                nc.tensor.matmul(ps_b[:], lhsT=ones_1row[:], rhs=rS[:],
                                 start=True, stop=True)
                nc.vector.tensor_copy(rSb[:], ps_b[:])
                for lc in range(LC):
                    nc.vector.tensor_tensor(attnT[:, lc, :], eAll[:, lc, :], rSb[:],
                                            OP.mult)

            if PHASE <= 7:
                return
            # ---- phase 9: outT[d, col] = V^T @ attnT, keep block-diagonal ----
            outTblk = pbig2.tile([P, KD, NSEL], BF16)
            nc.vector.memset(outTblk[:], 0.0)
            with tc.tile_pool(name="st9", bufs=3) as st9, \
                 tc.tile_pool(name="ps9", bufs=1, space="PSUM") as ps9:
                po = [ps9.tile([P, NSEL], F32, tag=f"po{dc}", name=f"po{dc}")
                      for dc in range(KD)]
                for lc in range(LC):
                    vrow = st9.tile([P, D], BF16, tag="vrow")
                    nc.sync.dma_start(vrow[:], v_bf[ts(lc, P), :])
                    for dc in range(KD):
                        nc.tensor.matmul(po[dc][:], lhsT=vrow[:, ts(dc, P)],
                                         rhs=attnT[:, lc, :],
                                         start=(lc == 0), stop=(lc == LC - 1))
                for dc in range(KD):
                    for hh in range(2):
                        h = 2 * dc + hh
                        nc.vector.tensor_copy(
                            outTblk[ds(hh * DK, DK), dc, ds(h * KMAX, KMAX)],
                            po[dc][ds(hh * DK, DK), ds(h * KMAX, KMAX)])

            if PHASE <= 9:
                return
            # ---- phase 10: y_sp[col, :] = outTblk.T @ WoT ----
            yA = pbig2.tile([NA, D], F32)
            yB = pbig2.tile([NB, D], F32)
            with tc.tile_pool(name="ps10", bufs=2, space="PSUM") as ps10:
                for rows, y_t, c0 in ((NA, yA, 0), (NB, yB, NA)):
                    for n in range(2):
                        py = ps10.tile([rows, 512], F32, tag=f"py{rows}")
                        for dc in range(KD):
                            nc.tensor.matmul(py[:], lhsT=outTblk[:, dc, ds(c0, rows)],
                                             rhs=wot_sb[:, dc, ts(n, 512)],
                                             start=(dc == 0), stop=(dc == KD - 1))
                        nc.vector.tensor_copy(y_t[:, ts(n, 512)], py[:])

            # ---- phase 11: merge colliding scatter rows via Eq matmul ----
            idxT = pconst.tile([P, NSEL], F32)
            eqA = pconst.tile([NA, NSEL], F32)
            eqB = pconst.tile([NB, NSEL], F32)
            ycA = pbig2.tile([NA, D], F32)
            ycB = pbig2.tile([NB, D], F32)
            with tc.tile_pool(name="ps11", bufs=2, space="PSUM") as ps11:
                piA = ps11.tile([P, NA], F32, tag="piA")
                nc.tensor.transpose(piA[:], sfA[:, 0:1].to_broadcast([NA, P]),
                                    ident_f[:NA, :NA])
                nc.vector.tensor_copy(idxT[:, 0:NA], piA[:])
                piB = ps11.tile([P, NB], F32, tag="piB")
                nc.tensor.transpose(piB[:], sfB[:, 0:1].to_broadcast([NB, P]),
                                    ident_f[:NB, :NB])
                nc.vector.tensor_copy(idxT[:, NA:NSEL], piB[:])
                nc.vector.tensor_tensor(eqA[:], sfA[:, 0:1].to_broadcast([NA, NSEL]),
                                        idxT[:NA, :], OP.is_equal)
                nc.vector.tensor_tensor(eqB[:], sfB[:, 0:1].to_broadcast([NB, NSEL]),
                                        idxT[:NB, :], OP.is_equal)
                for rows, yc_t, c0 in ((NA, ycA, 0), (NB, ycB, NA)):
                    for n in range(2):
                        pc = ps11.tile([rows, 512], F32, tag=f"pc{rows}")
                        nc.tensor.matmul(pc[:], lhsT=eqA[:, ds(c0, rows)],
                                         rhs=yA[:, ts(n, 512)], start=True, stop=False)
                        nc.tensor.matmul(pc[:], lhsT=eqB[:, ds(c0, rows)],
                                         rhs=yB[:, ts(n, 512)], start=False, stop=True)
                        # scattered rows replace the bo-base row -> include bo here
                        nc.vector.tensor_tensor(yc_t[:, ts(n, 512)], pc[:],
                                                bob[:rows, ts(n, 512)], OP.add)

            if DEBUG_DUMPS:
                for nm, t in (("d_qsTatt", qsT_att), ("d_S", S_sb), ("d_rSb", rSb),
                              ("d_yA", yA), ("d_yB", yB), ("d_ycA", ycA),
                              ("d_ycB", ycB), ("d_sfA", sfA), ("d_sfB", sfB),
                              ("d_eq", eqA), ("d_outTblk", outTblk),
                              ("d_attnT", attnT), ("d_eAll", eAll)):
                    dt = nc.dram_tensor(nm, list(t.shape), t[:].dtype).ap()
                    nc.sync.dma_start(dt[:], t[:])

            if PHASE <= 11:
                return
            # ---- phase 12: scatter (drop rows with idx == L via bounds_check) ----
            nc.gpsimd.indirect_dma_start(
                out=out[:], out_offset=bass.IndirectOffsetOnAxis(ap=siA[:, :1], axis=0),
                in_=ycA[:], in_offset=None, bounds_check=L - 1, oob_is_err=False)
            nc.gpsimd.indirect_dma_start(
                out=out[:], out_offset=bass.IndirectOffsetOnAxis(ap=siB[:, :1], axis=0),
                in_=ycB[:], in_offset=None, bounds_check=L - 1, oob_is_err=False)


_CACHE = {}


def get_nc(L=L_FULL):
    global _CACHE_KEY
    key = (L, PHASE, DEBUG_DUMPS)
    if key != globals().get("_CACHE_KEY") :
        _CACHE.clear()
        globals()["_CACHE_KEY"] = key
    if L not in _CACHE:
        nc = bacc.Bacc("TRN2", target_bir_lowering=False, debug=False)
        with tile.TileContext(nc) as tc:
            build_kernel(nc, tc, L)
        nc.compile()
        _CACHE[L] = nc
    return _CACHE[L]


def make_consts():
    rowbase = ((np.arange(P) // H) * P).astype(np.float32).reshape(P, 1)
    hvec = np.arange(H, dtype=np.float32).reshape(H, 1)
    slotA = (np.arange(NA) % KMAX).astype(np.float32).reshape(NA, 1)
    slotB = (np.arange(NB) % KMAX).astype(np.float32).reshape(NB, 1)
    return rowbase, hvec, slotA, slotB


def make_in_maps(x, Wq, bq, Wk, bk, Wv, bv, Wo, bo, L=L_FULL):
    rowbase, hvec, slotA, slotB = make_consts()
    shared = {
        "wqt": np.ascontiguousarray(Wq.T), "wkt": np.ascontiguousarray(Wk.T),
        "wvt": np.ascontiguousarray(Wv.T), "wot": np.ascontiguousarray(Wo.T),
        "bq": bq.reshape(1, D),
        "bkc": np.ascontiguousarray(bk.reshape(D // P, P).T),
        "bv": bv.reshape(1, D), "bo": bo.reshape(1, D),
        "rowbase": rowbase, "hvec16": hvec, "slotA": slotA, "slotB": slotB,
    }
    return [{"xt": np.ascontiguousarray(x[b, :L].T), **shared}
            for b in range(x.shape[0])]


def kernel(x, Wq, bq, Wk, bk, Wv, bv, Wo, bo):
    global LAST_RESULTS
    nc = get_nc(L_FULL)
    args = [np.asarray(a, np.float32) for a in (x, Wq, bq, Wk, bk, Wv, bv, Wo, bo)]
    in_maps = make_in_maps(*args)
    res = run_bass_kernel_spmd(
        nc, in_maps, core_ids=list(range(B)),
        trace=bool(int(os.environ.get("KERNEL_TRACE", "0"))))
    LAST_RESULTS = res
    return np.stack([res.results[i]["out"] for i in range(B)], axis=0)

